# revision 1
# baseline (speedup 1.0000x reference)
"""Trainium2 Bass kernel for nn_Encoder_70781061038947.

Math: row b's output depends on x[b, :] only through its 16 sign bits
(root k has radius R if x[b,k] > 0 else 1/R, phase shuffle_vector[k]).
The monic degree-16 polynomial is a product of three sub-polynomials over
bit-groups (6+5+5 bits).  Evaluate each group's sub-polynomial at the 17th
roots of unity via a one-hot matmul against a tiny table (64/32/32 rows),
multiply the three evaluations pointwise per row, normalize via Parseval
(||coeffs||^2 = mean |P(t_m)|^2), and interpolate coefficients back with a
17-point inverse-DFT matmul.  All O(B) work runs on-device:

  PE : sign transposes, one-hot match matmuls (bf16), table-gather matmuls
       (split-precision bf16 hi+lo), eval transposes, inverse-DFT matmuls
  ACT: sign extraction, one-hot thresholding relu(count + bias), eval
       staging copy, sqrt for the norm factor
  DVE/GPSIMD: pointwise complex products, norm, PSUM->SBUF moves

Sharding: pure data parallel over B across 8 cores (32768 rows each); the
small tables derived from shuffle_vector (host FLOPs independent of B) are
replicated inputs.
"""

import numpy as np
import ml_dtypes

import concourse.bacc as bacc
import concourse.bass as bass
import concourse.mybir as mybir
import concourse.bass_utils as bass_utils
import concourse.tile as tile

B = 262144
K = 16
M = 17                      # evaluation points (17th roots of unity)
W = 2 * M                   # 34 f32 per output row
NCORES = 8
RPC = B // NCORES           # 32768 rows per core
P = 128
CPB = RPC // P              # 256 rows per partition
TPC = 8                     # tiles (row-columns) per chunk
NCHUNK = CPB // TPC         # 32 chunks
GROUPS = [(0, 6), (6, 5), (11, 5)]   # (base bit, size): one-hot rows 64+32+32 = 128

_cached = None


def _tables(shuffle_vector: np.ndarray):
    sv = np.asarray(shuffle_vector, dtype=np.float64)
    R = np.sqrt(1.0 + np.sin(np.pi / K))
    t = np.exp(2j * np.pi * np.arange(M) / M)
    bf16 = ml_dtypes.bfloat16

    tbl = np.zeros((P, 3 * W), np.float64)   # [(g,nu), 34g + re/im]
    w3 = np.zeros((K, P), np.float64)
    biasv = np.zeros((P, 1), np.float64)
    row = 0
    for g, (base, size) in enumerate(GROUPS):
        for nu in range(1 << size):
            E = np.ones(M, np.complex128)
            for j in range(size):
                b = (nu >> j) & 1
                zk = (R if b else 1.0 / R) * np.exp(1j * sv[base + j])
                E = E * (t - zk)
            tbl[row, W * g: W * g + M] = E.real
            tbl[row, W * g + M: W * g + W] = E.imag
            for j in range(size):
                w3[base + j, row] = 2.0 * ((nu >> j) & 1) - 1.0
            # signs are +-1 on device: full match <=> dot == size
            biasv[row, 0] = 1 - size
            row += 1
    assert row == P

    # split-precision eval table: tbl ~= hi + lo with both halves bf16
    tbl_hi = tbl.astype(bf16)
    tbl_lo = (tbl - tbl_hi.astype(np.float64)).astype(bf16)

    w2r = np.zeros((W, W), np.float64)       # [re17||im17, interleaved re/im out]
    for m in range(M):
        for d in range(M):
            w = np.exp(-2j * np.pi * ((K - d) * m) / M) / M
            w2r[m, 2 * d] = w.real
            w2r[m, 2 * d + 1] = w.imag
            w2r[M + m, 2 * d] = -w.imag
            w2r[M + m, 2 * d + 1] = w.real

    # block-diagonal variants: 3-tile (102x102) and 2-tile (68x68) groups
    w2r3 = np.zeros((3 * W, 3 * W), np.float64)
    for j in range(3):
        w2r3[j * W:(j + 1) * W, j * W:(j + 1) * W] = w2r
    w2r2 = np.zeros((2 * W, 2 * W), np.float64)
    for j in range(2):
        w2r2[j * W:(j + 1) * W, j * W:(j + 1) * W] = w2r

    ident_bf = np.eye(P, dtype=bf16)
    ident_f = np.eye(P, dtype=np.float32)

    return {
        "w3": w3.astype(bf16),
        "biasv": biasv.astype(np.float32),
        "tblhi": tbl_hi,
        "tbllo": tbl_lo,
        "w2r3": w2r3.astype(np.float32),
        "w2r2": w2r2.astype(np.float32),
        "identb": ident_bf,
        "identf": ident_f,
    }


def _build_module(rpc=RPC):
    cpb = rpc // P
    nchunk = cpb // TPC
    f32 = mybir.dt.float32
    bf = mybir.dt.bfloat16
    FT = TPC * K             # 128: free width of one chunk of x
    FO = TPC * W             # 272: free width of one chunk of out
    AF = mybir.ActivationFunctionType
    OP = mybir.AluOpType

    nc = bacc.Bacc("TRN2", target_bir_lowering=False, debug=False)
    x_d = nc.dram_tensor("x", [rpc, K], bf, kind="ExternalInput")
    w3_d = nc.dram_tensor("w3", [K, P], bf, kind="ExternalInput")
    bias_d = nc.dram_tensor("biasv", [P, 1], f32, kind="ExternalInput")
    tblhi_d = nc.dram_tensor("tblhi", [P, 3 * W], bf, kind="ExternalInput")
    tbllo_d = nc.dram_tensor("tbllo", [P, 3 * W], bf, kind="ExternalInput")
    w2r3_d = nc.dram_tensor("w2r3", [3 * W, 3 * W], f32, kind="ExternalInput")
    w2r2_d = nc.dram_tensor("w2r2", [2 * W, 2 * W], f32, kind="ExternalInput")
    identb_d = nc.dram_tensor("identb", [P, P], bf, kind="ExternalInput")
    identf_d = nc.dram_tensor("identf", [P, P], f32, kind="ExternalInput")
    out_d = nc.dram_tensor("out", [rpc, W], f32, kind="ExternalOutput")

    # row (p*cpb + c) -> partition p, column c
    x_v = x_d.ap().rearrange("(p c) k -> p (c k)", p=P)      # [128, cpb*16]
    out_v = out_d.ap().rearrange("(p c) e -> p (c e)", p=P)  # [128, cpb*34]

    with tile.TileContext(nc) as tc:
        with (
            tc.tile_pool(name="const", bufs=1) as cp,
            tc.tile_pool(name="sb", bufs=4) as sp,
            tc.tile_pool(name="ps", bufs=1, space="PSUM") as pp,
        ):
            w3_sb = cp.tile([K, P], bf)
            nc.sync.dma_start(out=w3_sb[:], in_=w3_d.ap())
            bias_sb = cp.tile([P, 1], f32)
            nc.sync.dma_start(out=bias_sb[:], in_=bias_d.ap())
            tblhi_sb = cp.tile([P, 3 * W], bf)
            nc.sync.dma_start(out=tblhi_sb[:], in_=tblhi_d.ap())
            tbllo_sb = cp.tile([P, 3 * W], bf)
            nc.sync.dma_start(out=tbllo_sb[:], in_=tbllo_d.ap())
            w2r3_sb = cp.tile([3 * W, 3 * W], f32)
            nc.sync.dma_start(out=w2r3_sb[:], in_=w2r3_d.ap())
            w2r2_sb = cp.tile([2 * W, 2 * W], f32)
            nc.sync.dma_start(out=w2r2_sb[:], in_=w2r2_d.ap())
            identb = cp.tile([P, P], bf)
            nc.sync.dma_start(out=identb[:], in_=identb_d.ap())
            identf = cp.tile([P, P], f32)
            nc.sync.dma_start(out=identf[:], in_=identf_d.ap())

            for ci in range(nchunk):
                x_sb = sp.tile([P, FT], bf, tag="x")
                nc.sync.dma_start(out=x_sb[:], in_=x_v[:, ci * FT:(ci + 1) * FT])

                # per-tile transposes into one [16, 8*128] PSUM tile, then one
                # Sign: s_big[k, t*128+p] = sign(x of tile t row p), +-1 bf16
                xT = pp.tile([K, TPC * P], bf, tag="xT", bufs=2)
                for t in range(TPC):
                    nc.tensor.transpose(
                        out=xT[:, t * P:(t + 1) * P],
                        in_=x_sb[:, t * K:(t + 1) * K],
                        identity=identb[:])
                s_big = sp.tile([K, TPC * P], bf, tag="sbig")
                nc.scalar.activation(out=s_big[:], in_=xT[:], func=AF.Sign)

                # match counts: one merged matmul pair (K=16, N=512 each)
                mt = pp.tile([P, TPC * P], f32, tag="mtvr")
                for h in range(2):
                    nc.tensor.matmul(
                        out=mt[:, h * 512:(h + 1) * 512],
                        lhsT=w3_sb[:],
                        rhs=s_big[:, h * 512:(h + 1) * 512],
                        start=True, stop=True)

                ohT = sp.tile([P, TPC * P], bf, tag="ohT")
                nc.scalar.activation(
                    out=ohT[:], in_=mt[:], func=AF.Relu, bias=bias_sb[:], scale=1.0)

                # gather: per tile, split-precision bf16 hi+lo accumulated
                vr = pp.tile([P, TPC * P], f32, tag="vr")
                for t in range(TPC):
                    nc.tensor.matmul(
                        out=vr[:, t * P: t * P + 3 * W],
                        lhsT=ohT[:, t * P:(t + 1) * P],
                        rhs=tblhi_sb[:],
                        start=True, stop=False)
                    nc.tensor.matmul(
                        out=vr[:, t * P: t * P + 3 * W],
                        lhsT=ohT[:, t * P:(t + 1) * P],
                        rhs=tbllo_sb[:],
                        start=False, stop=True)

                # stage all evals into SBUF, packed 102 per tile
                ev_sb = sp.tile([P, TPC * 3 * W], f32, tag="evsb")
                evv = ev_sb[:].rearrange("p (t e) -> p t e", e=3 * W)
                nc.scalar.activation(
                    out=evv,
                    in_=vr[:].rearrange("p (t e) -> p t e", e=P)[:, :, 0:3 * W],
                    func=AF.Copy)
                e1r, e1i = evv[:, :, 0:M], evv[:, :, M:W]
                e2r, e2i = evv[:, :, W:W + M], evv[:, :, W + M:2 * W]
                e3r, e3i = evv[:, :, 2 * W:2 * W + M], evv[:, :, 2 * W + M:3 * W]

                def mk(tag):
                    return sp.tile([P, TPC * M], f32, tag=tag, name=tag)

                t1, t2, t3, t4 = mk("t1"), mk("t2"), mk("t3"), mk("t4")
                TR, TI = mk("TR"), mk("TI")
                t1v = t1[:].rearrange("p (t e) -> p t e", e=M)
                t2v = t2[:].rearrange("p (t e) -> p t e", e=M)
                t3v = t3[:].rearrange("p (t e) -> p t e", e=M)
                t4v = t4[:].rearrange("p (t e) -> p t e", e=M)
                nc.vector.tensor_tensor(out=t1v, in0=e1r, in1=e2r, op=OP.mult)
                nc.vector.tensor_tensor(out=t2v, in0=e1i, in1=e2i, op=OP.mult)
                nc.vector.tensor_tensor(out=t3v, in0=e1r, in1=e2i, op=OP.mult)
                nc.vector.tensor_tensor(out=t4v, in0=e1i, in1=e2r, op=OP.mult)
                nc.gpsimd.tensor_tensor(out=TR[:], in0=t1[:], in1=t2[:], op=OP.subtract)
                nc.gpsimd.tensor_tensor(out=TI[:], in0=t3[:], in1=t4[:], op=OP.add)

                u1, u2, u3, u4 = mk("u1"), mk("u2"), mk("u3"), mk("u4")
                TRv = TR[:].rearrange("p (t e) -> p t e", e=M)
                TIv = TI[:].rearrange("p (t e) -> p t e", e=M)
                u1v = u1[:].rearrange("p (t e) -> p t e", e=M)
                u2v = u2[:].rearrange("p (t e) -> p t e", e=M)
                u3v = u3[:].rearrange("p (t e) -> p t e", e=M)
                u4v = u4[:].rearrange("p (t e) -> p t e", e=M)
                nc.vector.tensor_tensor(out=u1v, in0=TRv, in1=e3r, op=OP.mult)
                nc.vector.tensor_tensor(out=u2v, in0=TIv, in1=e3i, op=OP.mult)
                nc.vector.tensor_tensor(out=u3v, in0=TRv, in1=e3i, op=OP.mult)
                nc.vector.tensor_tensor(out=u4v, in0=TIv, in1=e3r, op=OP.mult)

                # VC layout [128, (t), re17||im17] packed 34 per tile
                vc = sp.tile([P, FO], f32, tag="vc")
                vcv = vc[:].rearrange("p (t e) -> p t e", e=W)
                nc.gpsimd.tensor_tensor(
                    out=vcv[:, :, 0:M], in0=u1v, in1=u2v, op=OP.subtract)
                nc.gpsimd.tensor_tensor(
                    out=vcv[:, :, M:W], in0=u3v, in1=u4v, op=OP.add)

                sq = sp.tile([P, FO], f32, tag="sq")
                sqv = sq[:].rearrange("p (t e) -> p t e", e=W)
                nc.gpsimd.tensor_tensor(out=sqv, in0=vcv, in1=vcv, op=OP.mult)
                S = sp.tile([P, TPC], f32, tag="S")
                nc.vector.tensor_reduce(
                    out=S[:], in_=sqv, axis=mybir.AxisListType.X, op=OP.add)
                rS = sp.tile([P, TPC], f32, tag="rS")
                nc.vector.reciprocal(out=rS[:], in_=S[:])
                fac = sp.tile([P, TPC], f32, tag="fac")
                nc.scalar.activation(
                    out=fac[:], in_=rS[:], func=AF.Sqrt, bias=0.0, scale=float(M * M))
                nc.vector.tensor_tensor(
                    out=vcv, in0=vcv,
                    in1=fac[:].unsqueeze(2).to_broadcast([P, TPC, W]),
                    op=OP.mult)

                # transpose evals in tile-groups of (3,3,2); all operands base 0
                vcT = pp.tile([3 * W, 3 * P], f32, tag="vcT")
                widths = [3 * W, 3 * W, 2 * W]
                for j, wdt in enumerate(widths):
                    nc.tensor.transpose(
                        out=vcT[0:wdt, j * P:(j + 1) * P],
                        in_=vc[:, j * 3 * W: j * 3 * W + wdt],
                        identity=identf[:])
                vcT_sb = sp.tile([3 * W, 3 * P], f32, tag="vcTs")
                nc.vector.tensor_copy(out=vcT_sb[:], in_=vcT[:])

                # block-diagonal inverse-DFT: one matmul per tile-group
                o_ps = pp.tile([P, FO], f32, tag="o")
                nc.tensor.matmul(
                    out=o_ps[:, 0:3 * W], lhsT=vcT_sb[0:3 * W, 0:P],
                    rhs=w2r3_sb[:], start=True, stop=True)
                nc.tensor.matmul(
                    out=o_ps[:, 3 * W:6 * W], lhsT=vcT_sb[0:3 * W, P:2 * P],
                    rhs=w2r3_sb[:], start=True, stop=True)
                nc.tensor.matmul(
                    out=o_ps[:, 6 * W:8 * W], lhsT=vcT_sb[0:2 * W, 2 * P:3 * P],
                    rhs=w2r2_sb[:], start=True, stop=True)

                out_sb = sp.tile([P, FO], f32, tag="osb")
                nc.vector.tensor_copy(out=out_sb[:], in_=o_ps[:])
                nc.scalar.dma_start(
                    out=out_v[:, ci * FO:(ci + 1) * FO], in_=out_sb[:])

    nc.compile()
    return nc


def kernel(x: np.ndarray, shuffle_vector: np.ndarray) -> np.ndarray:
    global _cached
    x = np.asarray(x)
    assert x.shape == (B, K), x.shape
    x_bf = np.ascontiguousarray(x.astype(ml_dtypes.bfloat16))

    tabs = _tables(shuffle_vector)
    if _cached is None:
        _cached = _build_module()
    nc = _cached

    shards = x_bf.reshape(NCORES, RPC, K)
    in_maps = [
        {"x": np.ascontiguousarray(shards[i]), **tabs}
        for i in range(NCORES)
    ]
    res = bass_utils.run_bass_kernel_spmd(nc, in_maps, core_ids=list(range(NCORES)))
    out = np.concatenate([res.results[i]["out"] for i in range(NCORES)], axis=0)
    return np.ascontiguousarray(out).view(np.complex64).reshape(B, M).astype(np.complex128)



# revision 27
# speedup vs baseline: 1.0638x; 1.0638x over previous
"""Trainium2 Bass kernel for nn_Encoder_70781061038947.

Math: row b's output depends on x[b, :] only through its 16 sign bits
(root k has radius R if x[b,k] > 0 else 1/R, phase shuffle_vector[k]).
P_b(t) = prod_k (t - z_k) is monic of degree 16, so its 17 coefficients are
determined by the 16 values P_b(t_m) at the 16th roots of unity t_m plus
c_0 = 1.  Split the 16 bits into four 4-bit groups; per group precompute a
16-entry table of (log|E_g(t_m)|, arg E_g(t_m)) on the host (O(1) work).

Device pipeline per core (pure data parallel over B, 32768 rows/core):
  sign bits -> one-hot match counts (PE matmul, 64-row table, two chunks
  stacked in PSUM partition halves) -> one-hot (Act relu + DVE is_equal)
  -> gather log-mag/phase sums (PE matmul vs fp16 table, K=128 sums the 4
  groups in PSUM) -> E = exp(L) (Act) ... phase range-reduce mod 2pi
  (Pool, int32 round trick) -> sin / half-angle cos (Act) -> P = E*(c, s)
  (DVE/Pool) -> transpose 4 tiles at a time (PE) -> 16-point inverse DFT
  via block-diagonal(W2 x4) fp16 matmul (PE) -> q (banded, fp16) -> HBM.

Host finishes with O(B) numpy: c_16 = q_0 - 1, c_d = q_{16-d}, Parseval
norm l2^2 = 1 + |q_0 - 1|^2 + sum_{e>=1} |q_e|^2, scale by sqrt(17)/l2.
Two activation-table phases (exp set, then trig set) avoid ACT_TABLE_LOAD
thrash.
"""

import numpy as np
import ml_dtypes

import concourse.bacc as bacc
import concourse.bass as bass
import concourse.mybir as mybir
import concourse.bass_utils as bass_utils
import concourse.tile as tile

B = 262144
K = 16
M = 16                       # eval points: 16th roots of unity
NCORES = 8
RPC = B // NCORES            # 32768 rows per core
P = 128
NBLK = 8                     # row blocks per core (4096 rows each)
BLKC = RPC // NBLK           # 4096 cols per block
NG = 4                       # bit groups
GS = 4                       # bits per group
TROWS = NG * (1 << GS)       # 64 table rows
TW = 2 * M                   # 32 table cols: L0..15 | A0..15

f32 = mybir.dt.float32
f16 = mybir.dt.float16
bf16 = mybir.dt.bfloat16
i32 = mybir.dt.int32
AF = mybir.ActivationFunctionType
OP = mybir.AluOpType

_cached = None


def _tables(shuffle_vector: np.ndarray):
    sv = np.asarray(shuffle_vector, dtype=np.float64)
    R = np.sqrt(1.0 + np.sin(np.pi / K))
    t = np.exp(2j * np.pi * np.arange(M) / M)
    fp16 = np.float16

    # per-group log-mag/phase tables; table row r = 16*g + nu
    tbl = np.zeros((TROWS, TW), np.float64)
    w3 = np.zeros((K, TROWS), np.float64)      # {0,1}-sign match weights
    n1 = np.zeros(TROWS, np.float64)
    for g in range(NG):
        for nu in range(1 << GS):
            r = 16 * g + nu
            E = np.ones(M, np.complex128)
            for j in range(GS):
                b = (nu >> j) & 1
                zk = (R if b else 1.0 / R) * np.exp(1j * sv[4 * g + j])
                E = E * (t - zk)
                w3[4 * g + j, r] = 2.0 * b - 1.0
            # -1 per group keeps exp(sum L) < 2600, inside fp16 range;
            # the uniform e^4 factor is restored on the host.
            tbl[r, 0:M] = np.log(np.abs(E)) - 1.0
            ang = np.angle(E)  # in (-pi, pi]
            tbl[r, M:TW] = ang
            n1[r] = bin(nu).count("1")

    # w3stack [128, 4*128]: K=128 match weights, all matmuls at PE tile
    # (0,0) — mixing tile positions between matmuls faults the hardware.
    # Variant a (pair = blocks a, a+4): out cols 0-63 = block a's table
    # (w3 on rows 16a..16a+16), cols 64-127 = block a+4's (rows 64+16a..).
    w3stack = np.zeros((P, 4 * P), np.float64)
    for am in range(4):
        w3stack[16 * am:16 * am + 16, P * am:P * am + TROWS] = w3
        w3stack[64 + 16 * am:64 + 16 * am + 16,
                P * am + TROWS:P * (am + 1)] = w3
    biasv = np.zeros((P, 1), np.float32)
    sizev = np.zeros((P, 1), np.float32)
    for q in range(P):
        biasv[q, 0] = 1.0 - n1[q % TROWS]
        sizev[q, 0] = n1[q % TROWS]

    # zero-padded K=128 gather tables: cols 0:32 for partition-half A
    # (rows 0-63 live), cols 32:64 for half B (rows 64-127 live)
    tbl2 = np.zeros((P, 2 * TW), np.float64)
    tbl2[0:TROWS, 0:TW] = tbl
    tbl2[TROWS:2 * TROWS, TW:2 * TW] = tbl

    # 16-pt inverse DFT in real form: in-comp (re0..15, im0..15) ->
    # out-comp (2e: Re q_e, 2e+1: Im q_e), q_e = (1/16) sum_m Q_m w^{-me}
    W2 = np.zeros((TW, TW), np.float64)
    for m in range(M):
        for e in range(M):
            w = np.exp(-2j * np.pi * m * e / M) / M
            W2[m, 2 * e] = w.real
            W2[m, 2 * e + 1] = w.imag
            W2[M + m, 2 * e] = -w.imag
            W2[M + m, 2 * e + 1] = w.real
    bdw2 = np.zeros((P, P), np.float64)
    for j in range(4):
        bdw2[TW * j:TW * (j + 1), TW * j:TW * (j + 1)] = W2

    return {
        "w3stack": w3stack.astype(fp16),
        "biasv": biasv,
        "sizev": sizev,
        "tbl2": tbl2.astype(fp16),
        "bdw2": bdw2.astype(fp16),
        "ident": np.eye(P, dtype=fp16),
    }


def _build_module(stage: int = 99):
    nc = bacc.Bacc("TRN2", target_bir_lowering=False, debug=False)
    x_d = nc.dram_tensor("xT8", [P, RPC * K // P], bf16, kind="ExternalInput")
    assert RPC * K // P == 4096
    w3_d = nc.dram_tensor("w3stack", [P, 4 * P], f16, kind="ExternalInput")
    biasv_d = nc.dram_tensor("biasv", [P, 1], f32, kind="ExternalInput")
    sizev_d = nc.dram_tensor("sizev", [P, 1], f32, kind="ExternalInput")
    tbl_d = nc.dram_tensor("tbl2", [P, 2 * TW], f16, kind="ExternalInput")
    bdw2_d = nc.dram_tensor("bdw2", [P, P], f16, kind="ExternalInput")
    ident_d = nc.dram_tensor("ident", [P, P], f16, kind="ExternalInput")
    q_d = nc.dram_tensor("q", [P, 8192], f16, kind="ExternalOutput")

    XCOLS = 4096             # xT8 free size
    NGRP = 8                 # 2-pair groups, 4096 rows each
    GW = 32 * TW             # 1024: vr/vc cols per group (32 tiles x 32)
    INV2PI = float(1.0 / (2 * np.pi))
    TWOPI = float(2 * np.pi)

    with tile.TileContext(nc) as tc:
        with (
            tc.tile_pool(name="const", bufs=1) as cp,
            tc.tile_pool(name="sb", bufs=2) as sp,
            tc.tile_pool(name="ps", bufs=1, space="PSUM") as pp,
        ):
            w3s = cp.tile([P, 4 * P], f16)
            nc.sync.dma_start(out=w3s[:], in_=w3_d.ap())
            biasv = cp.tile([P, 1], f32)
            nc.sync.dma_start(out=biasv[:], in_=biasv_d.ap())
            sizev = cp.tile([P, 1], f32)
            nc.sync.dma_start(out=sizev[:], in_=sizev_d.ap())
            tbl = cp.tile([P, 2 * TW], f16)
            nc.sync.dma_start(out=tbl[:], in_=tbl_d.ap())
            bdw2 = cp.tile([P, P], f16)
            nc.sync.dma_start(out=bdw2[:], in_=bdw2_d.ap())
            ident = cp.tile([P, P], f16)
            nc.sync.dma_start(out=ident[:], in_=ident_d.ap())

            xT8 = cp.tile([P, XCOLS], bf16)
            nc.sync.dma_start(out=xT8[:], in_=x_d.ap())

            # persistent across passes
            s = cp.tile([P, XCOLS], f16, name="s")
            E_all = cp.tile([P, NGRP * 512], f16, name="E_all")
            A_all = cp.tile([P, NGRP * 512], f32, name="A_all")

            # signs as {1,0}: one op, feeds all match matmuls
            nc.gpsimd.tensor_scalar(
                out=s[:], in0=xT8[:], scalar1=0.0, scalar2=None, op0=OP.is_gt)

            # ---------------- PASS 1: match, one-hot, gather, exp ----------
            for G in range(NGRP):
                a, h = G // 2, G % 2      # pair = (block a, block a+4)
                for half in range(2):     # chunk pair within group
                    colbase = 2048 * h + 1024 * half
                    vr = pp.tile([P, GW // 2], f32, tag="vr", bufs=2)
                    wsl = w3s[:, P * a: P * (a + 1)]
                    for ch in range(2):   # 512-col halves keep mt in one bank
                        cb2 = colbase + 512 * ch
                        mt = pp.tile([P, 512], f32, tag="mt", bufs=2)
                        nc.tensor.matmul(
                            out=mt[:], lhsT=wsl,
                            rhs=s[:, cb2:cb2 + 512], start=True, stop=True)

                        ohh = sp.tile([P, 512], f16, tag="ohh")
                        nc.scalar.activation(
                            out=ohh[:, 0:256], in_=mt[:, 0:256], func=AF.Relu,
                            bias=biasv[:], scale=1.0)
                        nc.vector.tensor_tensor(
                            out=ohh[:, 256:512], in0=mt[:, 256:512],
                            in1=sizev[:].to_broadcast([P, 256]), op=OP.is_equal)

                        # K=128 gathers at tile (0,0); zero-padded table
                        # halves select the partition half
                        for bb in range(2):
                            for t in range(4):
                                tau = 8 * bb + 4 * ch + t
                                nc.tensor.matmul(
                                    out=vr[:, TW * tau: TW * (tau + 1)],
                                    lhsT=ohh[:, 128 * t:128 * (t + 1)],
                                    rhs=tbl[:, TW * bb: TW * (bb + 1)],
                                    start=True, stop=True)

                    p2 = 2 * G + half
                    vrv = vr[:].rearrange("p (t e) -> p t e", e=TW)
                    nc.scalar.activation(
                        out=E_all[:, 256 * p2: 256 * (p2 + 1)].rearrange(
                            "p (t e) -> p t e", e=M),
                        in_=vrv[:, :, 0:M], func=AF.Exp)
                    nc.scalar.activation(
                        out=A_all[:, 256 * p2: 256 * (p2 + 1)].rearrange(
                            "p (t e) -> p t e", e=M),
                        in_=vrv[:, :, M:TW], func=AF.Copy)

            if stage == 1:
                # debug: dump E_all/A_all via q (reinterpret cols)
                eq = sp.tile([P, 512], f16, tag="eqd")
                for G in range(NGRP):
                    nc.vector.tensor_copy(out=eq[:], in_=E_all[:, 512 * G:512 * (G + 1)])
                    nc.scalar.dma_start(
                        out=q_d.ap()[:, 1024 * G:1024 * G + 512], in_=eq[:])
                    eq2 = sp.tile([P, 512], f16, tag="eqd2")
                    nc.vector.tensor_copy(out=eq2[:], in_=A_all[:, 512 * G:512 * (G + 1)])
                    nc.scalar.dma_start(
                        out=q_d.ap()[:, 1024 * G + 512:1024 * (G + 1)], in_=eq2[:])

            # ---------------- PASS 2: trig, assemble, transpose, iDFT ------
            for G in range(NGRP if stage >= 2 else 0):
                Ag = A_all[:, 512 * G: 512 * (G + 1)]
                Eg = E_all[:, 512 * G: 512 * (G + 1)].rearrange(
                    "p (t e) -> p t e", e=M)

                # round(A/2pi) via the fp32 magic-number trick: adding
                # 1.5*2^23 forces RTNE to integer in any IEEE ALU, so
                # CoreSim and hardware agree (int casts don't).
                MAGIC = float(1.5 * 2 ** 23)
                t1 = sp.tile([P, 512], f32, tag="t1")
                nc.gpsimd.tensor_scalar(
                    out=t1[:], in0=Ag, scalar1=INV2PI, scalar2=MAGIC,
                    op0=OP.mult, op1=OP.add)
                k2p = sp.tile([P, 512], f32, tag="k2p")
                nc.gpsimd.tensor_scalar(
                    out=k2p[:], in0=t1[:], scalar1=MAGIC, scalar2=TWOPI,
                    op0=OP.subtract, op1=OP.mult)
                ar = sp.tile([P, 512], f32, tag="ar")
                nc.gpsimd.tensor_tensor(out=ar[:], in0=Ag, in1=k2p[:], op=OP.subtract)

                sA = sp.tile([P, 512], f16, tag="sA")
                nc.scalar.activation(out=sA[:], in_=ar[:], func=AF.Sin,
                                     scale=float(1.0 - 1e-6))
                hA = sp.tile([P, 512], f16, tag="hA")
                nc.scalar.activation(out=hA[:], in_=ar[:], func=AF.Sin,
                                     scale=float(0.5 * (1.0 - 1e-6)))
                m2 = sp.tile([P, 512], f16, tag="m2")
                nc.vector.tensor_tensor(out=m2[:], in0=hA[:], in1=hA[:], op=OP.mult)
                cA = sp.tile([P, 512], f16, tag="cA")
                nc.vector.tensor_scalar(
                    out=cA[:], in0=m2[:], scalar1=-2.0, scalar2=1.0,
                    op0=OP.mult, op1=OP.add)

                vc = sp.tile([P, GW], f16, tag="vc")
                vcv = vc[:].rearrange("p (t e) -> p t e", e=TW)
                sAv = sA[:].rearrange("p (t e) -> p t e", e=M)
                cAv = cA[:].rearrange("p (t e) -> p t e", e=M)
                nc.vector.tensor_tensor(out=vcv[:, :, 0:M], in0=Eg, in1=cAv, op=OP.mult)
                nc.gpsimd.tensor_tensor(out=vcv[:, :, M:TW], in0=Eg, in1=sAv, op=OP.mult)

                if stage == 2:
                    nc.scalar.dma_start(
                        out=q_d.ap()[:, 1024 * G:1024 * (G + 1)], in_=vc[:])
                    continue

                for half in range(2):
                    vcT = pp.tile([P, 512], f16, tag="vcT", bufs=2)
                    for t4 in range(4):
                        nc.tensor.transpose(
                            out=vcT[:, 128 * t4:128 * (t4 + 1)],
                            in_=vc[:, 512 * half + 128 * t4: 512 * half + 128 * (t4 + 1)],
                            identity=ident[:])
                    vcTs = sp.tile([P, 512], f16, tag="vcTs")
                    nc.vector.tensor_copy(out=vcTs[:], in_=vcT[:])
                    if stage == 3:
                        nc.scalar.dma_start(
                            out=q_d.ap()[:, 1024 * G + 512 * half:
                                         1024 * G + 512 * (half + 1)],
                            in_=vcTs[:])
                        continue
                    qT = pp.tile([P, 512], f32, tag="qT", bufs=2)
                    nc.tensor.matmul(out=qT[:], lhsT=bdw2[:], rhs=vcTs[:],
                                     start=True, stop=True)
                    qs = sp.tile([P, 512], f16, tag="qs")
                    if half == 0:
                        nc.scalar.activation(out=qs[:], in_=qT[:], func=AF.Copy)
                    else:
                        nc.vector.tensor_copy(out=qs[:], in_=qT[:])
                    nc.scalar.dma_start(
                        out=q_d.ap()[:, 1024 * G + 512 * half: 1024 * G + 512 * (half + 1)],
                        in_=qs[:])

    nc.compile()
    return nc


def _prep_inputs(x: np.ndarray, shuffle_vector: np.ndarray):
    x_bf = np.asarray(x).astype(ml_dtypes.bfloat16)
    tabs = _tables(shuffle_vector)
    in_maps = []
    for n in range(NCORES):
        xc = x_bf[n * RPC:(n + 1) * RPC]                    # [32768, 16]
        xT8 = np.ascontiguousarray(
            xc.reshape(NBLK, BLKC, K).transpose(0, 2, 1).reshape(P, BLKC))
        in_maps.append({"xT8": xT8, **tabs})
    return in_maps


def _decode(q_cores: list, shuffle_vector: np.ndarray) -> np.ndarray:
    # q_d [128, 8192] fp16 per core; partition 32j+dc, col 128*tg + l
    # tile tau = 4*(tg % 8) ... see mapping below
    out = np.empty((B, M), np.complex128)
    row_of = np.empty((4, 64, 128), np.int64)    # [j, tg, l] -> row in core
    for tg in range(64):
        G, t4 = tg // 8, tg % 8
        a, h = G // 2, G % 2
        for j in range(4):
            tau = 4 * t4 + j
            cc, t = tau // 8, tau % 8
            blk = a + 4 * (cc % 2)
            colib = 2048 * h + 1024 * (cc // 2) + 128 * t
            row_of[j, tg, :] = 4096 * blk + colib + np.arange(128)
    for n in range(NCORES):
        q = np.asarray(q_cores[n], np.float64).reshape(4, TW, 64, P)  # j, dc, tg, l
        qc = q[:, 0::2] + 1j * q[:, 1::2]                             # j, e, tg, l
        rows = np.empty((RPC, M), np.complex128)
        ro = row_of.reshape(4, 64 * 128)
        for j in range(4):
            rows[ro[j]] = qc[j].reshape(M, 64 * 128).T
        out[n * RPC:(n + 1) * RPC] = rows
    # coefficients: c_0 = 1, c_16 = q_0 - 1, c_d = q_{16-d} for d=1..15
    out *= np.exp(4.0)              # undo the per-group -1 log-mag offset
    q0 = out[:, 0]
    coeffs = np.empty((B, K + 1), np.complex128)
    coeffs[:, 0] = 1.0
    coeffs[:, 16] = q0 - 1.0
    coeffs[:, 1:16] = out[:, 1:][:, ::-1]
    l2 = np.sqrt(np.sum(np.abs(coeffs) ** 2, axis=-1, keepdims=True))
    return coeffs / l2 * np.sqrt(K + 1)


def kernel(x: np.ndarray, shuffle_vector: np.ndarray) -> np.ndarray:
    global _cached
    x = np.asarray(x)
    assert x.shape == (B, K), x.shape
    if _cached is None:
        _cached = _build_module()
    in_maps = _prep_inputs(x, shuffle_vector)
    res = bass_utils.run_bass_kernel_spmd(
        _cached, in_maps, core_ids=list(range(NCORES)))
    return _decode([res.results[n]["q"] for n in range(NCORES)], shuffle_vector)


# revision 35
# speedup vs baseline: 2.1405x; 2.0121x over previous
"""Trainium2 Bass kernel for nn_Encoder_70781061038947.

Math: row b's output depends on x[b, :] only through its 16 sign bits
(root k has radius R if x[b,k] > 0 else 1/R, phase shuffle_vector[k]).
P_b(t) = prod_k (t - z_k) is monic of degree 16, so its 17 coefficients are
determined by the 16 values P_b(t_m) at the 16th roots of unity t_m plus
c_0 = 1.  Split the 16 bits into four 4-bit groups; per group precompute a
16-entry table of (log|E_g(t_m)|, arg E_g(t_m)) on the host (O(1) work).

Device pipeline per core (pure data parallel over B, 32768 rows/core):
  sign bits -> one-hot match counts (PE matmul, 64-row table, two chunks
  stacked in PSUM partition halves) -> one-hot (Act relu + DVE is_equal)
  -> gather log-mag/phase sums (PE matmul vs fp16 table, K=128 sums the 4
  groups in PSUM) -> E = exp(L) (Act) ... phase range-reduce mod 2pi
  (Pool, int32 round trick) -> sin / half-angle cos (Act) -> P = E*(c, s)
  (DVE/Pool) -> transpose 4 tiles at a time (PE) -> 16-point inverse DFT
  via block-diagonal(W2 x4) fp16 matmul (PE) -> q (banded, fp16) -> HBM.

Host finishes with O(B) numpy: c_16 = q_0 - 1, c_d = q_{16-d}, Parseval
norm l2^2 = 1 + |q_0 - 1|^2 + sum_{e>=1} |q_e|^2, scale by sqrt(17)/l2.
Two activation-table phases (exp set, then trig set) avoid ACT_TABLE_LOAD
thrash.
"""

import numpy as np
import ml_dtypes

import concourse.bacc as bacc
import concourse.bass as bass
import concourse.mybir as mybir
import concourse.bass_utils as bass_utils
import concourse.tile as tile

B = 262144
K = 16
M = 16                       # eval points: 16th roots of unity
NCORES = 8
RPC = B // NCORES            # 32768 rows per core
P = 128
NBLK = 8                     # row blocks per core (4096 rows each)
BLKC = RPC // NBLK           # 4096 cols per block
NG = 4                       # bit groups
GS = 4                       # bits per group
TROWS = NG * (1 << GS)       # 64 table rows
TW = 2 * M                   # 32 table cols: L0..15 | A0..15

f32 = mybir.dt.float32
f16 = mybir.dt.float16
bf16 = mybir.dt.bfloat16
i32 = mybir.dt.int32
AF = mybir.ActivationFunctionType
OP = mybir.AluOpType

_cached = None


def _tables(shuffle_vector: np.ndarray):
    sv = np.asarray(shuffle_vector, dtype=np.float64)
    R = np.sqrt(1.0 + np.sin(np.pi / K))
    t = np.exp(2j * np.pi * np.arange(M) / M)
    fp16 = np.float16

    # per-group log-mag/phase tables; table row r = 16*g + nu
    tbl = np.zeros((TROWS, TW), np.float64)
    w3 = np.zeros((K, TROWS), np.float64)      # {0,1}-sign match weights
    n1 = np.zeros(TROWS, np.float64)
    for g in range(NG):
        for nu in range(1 << GS):
            r = 16 * g + nu
            E = np.ones(M, np.complex128)
            for j in range(GS):
                b = (nu >> j) & 1
                zk = (R if b else 1.0 / R) * np.exp(1j * sv[4 * g + j])
                E = E * (t - zk)
                w3[4 * g + j, r] = 2.0 * b - 1.0
            # -1 per group keeps exp(sum L) < 2600, inside fp16 range;
            # the uniform e^4 factor is restored on the host.
            tbl[r, 0:M] = np.log(np.abs(E)) - 1.0
            # phases stored in cycles (units of 2pi): range reduction is
            # then a round() via magic-bias adds, and Sin's scale converts
            tbl[r, M:TW] = np.angle(E) / (2 * np.pi)
            n1[r] = bin(nu).count("1")

    # w3stack [128, 4*128]: K=128 match weights, all matmuls at PE tile
    # (0,0) — mixing tile positions between matmuls faults the hardware.
    # Variant a (pair = blocks a, a+4): out cols 0-63 = block a's table
    # (w3 on rows 16a..16a+16), cols 64-127 = block a+4's (rows 64+16a..).
    w3stack = np.zeros((P, 4 * P), np.float64)
    for am in range(4):
        w3stack[16 * am:16 * am + 16, P * am:P * am + TROWS] = w3
        w3stack[64 + 16 * am:64 + 16 * am + 16,
                P * am + TROWS:P * (am + 1)] = w3
    # +-1 sign convention: match count = sum w~_j s_j = 4 iff all bits match
    biasv = np.full((P, 1), -3.0, np.float32)
    sizev = np.full((P, 1), 4.0, np.float32)

    # zero-padded K=128 gather tables: cols 0:32 for partition-half A
    # (rows 0-63 live), cols 32:64 for half B (rows 64-127 live)
    tbl2 = np.zeros((P, 2 * TW), np.float64)
    tbl2[0:TROWS, 0:TW] = tbl
    tbl2[TROWS:2 * TROWS, TW:2 * TW] = tbl

    # 16-pt inverse DFT in real form: in-comp (re0..15, im0..15) ->
    # out-comp (2e: Re q_e, 2e+1: Im q_e), q_e = (1/16) sum_m Q_m w^{-me}
    W2 = np.zeros((TW, TW), np.float64)
    for m in range(M):
        for e in range(M):
            w = np.exp(-2j * np.pi * m * e / M) / M
            W2[m, 2 * e] = w.real
            W2[m, 2 * e + 1] = w.imag
            W2[M + m, 2 * e] = -w.imag
            W2[M + m, 2 * e + 1] = w.real
    bdw2 = np.zeros((P, P), np.float64)
    for j in range(4):
        bdw2[TW * j:TW * (j + 1), TW * j:TW * (j + 1)] = W2

    return {
        "w3stack": w3stack.astype(fp16),
        "biasv": biasv,
        "sizev": sizev,
        "tbl2": tbl2.astype(fp16),
        "bdw2": bdw2.astype(fp16),
        "ident": np.eye(P, dtype=fp16),
        "magp": np.full((P, 1), 1.5 * 2 ** 23, np.float32),
        "magn": np.full((P, 1), -1.5 * 2 ** 23, np.float32),
    }


def _build_module(stage: int = 99):
    nc = bacc.Bacc("TRN2", target_bir_lowering=False, debug=False)
    x_d = nc.dram_tensor("xT8", [P, RPC * K // P], bf16, kind="ExternalInput")
    assert RPC * K // P == 4096
    w3_d = nc.dram_tensor("w3stack", [P, 4 * P], f16, kind="ExternalInput")
    biasv_d = nc.dram_tensor("biasv", [P, 1], f32, kind="ExternalInput")
    sizev_d = nc.dram_tensor("sizev", [P, 1], f32, kind="ExternalInput")
    tbl_d = nc.dram_tensor("tbl2", [P, 2 * TW], f16, kind="ExternalInput")
    bdw2_d = nc.dram_tensor("bdw2", [P, P], f16, kind="ExternalInput")
    ident_d = nc.dram_tensor("ident", [P, P], f16, kind="ExternalInput")
    magp_d = nc.dram_tensor("magp", [P, 1], f32, kind="ExternalInput")
    magn_d = nc.dram_tensor("magn", [P, 1], f32, kind="ExternalInput")
    q_d = nc.dram_tensor("q", [P, 8192], f16, kind="ExternalOutput")

    XCOLS = 4096             # xT8 free size
    NGRP = 8                 # 2-pair groups, 4096 rows each
    GW = 32 * TW             # 1024: vr/vc cols per group (32 tiles x 32)
    INV2PI = float(1.0 / (2 * np.pi))
    TWOPI = float(2 * np.pi)

    with tile.TileContext(nc) as tc:
        with (
            tc.tile_pool(name="const", bufs=1) as cp,
            tc.tile_pool(name="sb", bufs=2) as sp,
            tc.tile_pool(name="ps", bufs=1, space="PSUM") as pp,
        ):
            w3s = cp.tile([P, 4 * P], f16)
            nc.sync.dma_start(out=w3s[:], in_=w3_d.ap())
            biasv = cp.tile([P, 1], f32)
            nc.sync.dma_start(out=biasv[:], in_=biasv_d.ap())
            sizev = cp.tile([P, 1], f32)
            nc.sync.dma_start(out=sizev[:], in_=sizev_d.ap())
            tbl = cp.tile([P, 2 * TW], f16)
            nc.sync.dma_start(out=tbl[:], in_=tbl_d.ap())
            bdw2 = cp.tile([P, P], f16)
            nc.sync.dma_start(out=bdw2[:], in_=bdw2_d.ap())
            ident = cp.tile([P, P], f16)
            nc.sync.dma_start(out=ident[:], in_=ident_d.ap())
            magp = cp.tile([P, 1], f32)
            nc.sync.dma_start(out=magp[:], in_=magp_d.ap())
            magn = cp.tile([P, 1], f32)
            nc.sync.dma_start(out=magn[:], in_=magn_d.ap())

            xT8 = cp.tile([P, XCOLS], bf16)
            nc.sync.dma_start(out=xT8[:], in_=x_d.ap())

            # persistent across passes
            s = cp.tile([P, XCOLS], f16, name="s")
            E_all = cp.tile([P, NGRP * 512], f16, name="E_all")
            A_all = cp.tile([P, NGRP * 512], f32, name="A_all")

            # signs as +-1: one Act op (Pool tensor_scalar has a ~7us Q7
            # launch overhead per instruction -- keep Pool off the hot path)
            nc.scalar.activation(out=s[:], in_=xT8[:], func=AF.Sign)

            # ---------------- PASS 1: match, one-hot, gather, exp ----------
            for G in range(NGRP):
                a, h = G // 2, G % 2      # pair = (block a, block a+4)
                for half in range(2):     # chunk pair within group
                    colbase = 2048 * h + 1024 * half
                    vr = pp.tile([P, GW // 2], f32, tag="vr", bufs=2)
                    wsl = w3s[:, P * a: P * (a + 1)]
                    for ch in range(2):   # 512-col halves keep mt in one bank
                        cb2 = colbase + 512 * ch
                        mt = pp.tile([P, 512], f32, tag="mt", bufs=2)
                        nc.tensor.matmul(
                            out=mt[:], lhsT=wsl,
                            rhs=s[:, cb2:cb2 + 512], start=True, stop=True)

                        ohh = sp.tile([P, 512], f16, tag="ohh")
                        nc.scalar.activation(
                            out=ohh[:, 0:256], in_=mt[:, 0:256], func=AF.Relu,
                            bias=biasv[:], scale=1.0)
                        nc.vector.tensor_tensor(
                            out=ohh[:, 256:512], in0=mt[:, 256:512],
                            in1=sizev[:].to_broadcast([P, 256]), op=OP.is_equal)

                        # K=128 gathers at tile (0,0); zero-padded table
                        # halves select the partition half
                        for bb in range(2):
                            for t in range(4):
                                tau = 8 * bb + 4 * ch + t
                                nc.tensor.matmul(
                                    out=vr[:, TW * tau: TW * (tau + 1)],
                                    lhsT=ohh[:, 128 * t:128 * (t + 1)],
                                    rhs=tbl[:, TW * bb: TW * (bb + 1)],
                                    start=True, stop=True)

                    p2 = 2 * G + half
                    vrv = vr[:].rearrange("p (t e) -> p t e", e=TW)
                    nc.scalar.activation(
                        out=E_all[:, 256 * p2: 256 * (p2 + 1)].rearrange(
                            "p (t e) -> p t e", e=M),
                        in_=vrv[:, :, 0:M], func=AF.Exp)
                    nc.scalar.activation(
                        out=A_all[:, 256 * p2: 256 * (p2 + 1)].rearrange(
                            "p (t e) -> p t e", e=M),
                        in_=vrv[:, :, M:TW], func=AF.Copy)

            if stage == 1:
                # debug: dump E_all/A_all via q (reinterpret cols)
                eq = sp.tile([P, 512], f16, tag="eqd")
                for G in range(NGRP):
                    nc.vector.tensor_copy(out=eq[:], in_=E_all[:, 512 * G:512 * (G + 1)])
                    nc.scalar.dma_start(
                        out=q_d.ap()[:, 1024 * G:1024 * G + 512], in_=eq[:])
                    eq2 = sp.tile([P, 512], f16, tag="eqd2")
                    nc.vector.tensor_copy(out=eq2[:], in_=A_all[:, 512 * G:512 * (G + 1)])
                    nc.scalar.dma_start(
                        out=q_d.ap()[:, 1024 * G + 512:1024 * (G + 1)], in_=eq2[:])

            # ---------------- PASS 2: trig, assemble, transpose, iDFT ------
            for G in range(NGRP if stage >= 2 else 0):
                Ag = A_all[:, 512 * G: 512 * (G + 1)]
                Eg = E_all[:, 512 * G: 512 * (G + 1)].rearrange(
                    "p (t e) -> p t e", e=M)

                # A is in cycles; k = round(A) via the fp32 magic-bias trick
                # (RTNE in any IEEE ALU, identical in CoreSim and hardware)
                t1 = sp.tile([P, 512], f32, tag="t1")
                nc.scalar.activation(out=t1[:], in_=Ag, func=AF.Copy,
                                     bias=float(1.5 * 2 ** 23), scale=1.0)
                kk = sp.tile([P, 512], f32, tag="kk")
                nc.scalar.activation(out=kk[:], in_=t1[:], func=AF.Copy,
                                     bias=float(-1.5 * 2 ** 23), scale=1.0)
                ar = sp.tile([P, 512], f32, tag="ar")
                nc.vector.tensor_tensor(out=ar[:], in0=Ag, in1=kk[:], op=OP.subtract)

                sA = sp.tile([P, 512], f16, tag="sA")
                nc.scalar.activation(out=sA[:], in_=ar[:], func=AF.Sin,
                                     scale=float(2 * np.pi * (1.0 - 1e-6)))
                hA = sp.tile([P, 512], f16, tag="hA")
                nc.scalar.activation(out=hA[:], in_=ar[:], func=AF.Sin,
                                     scale=float(np.pi * (1.0 - 1e-6)))
                m2 = sp.tile([P, 512], f16, tag="m2")
                nc.vector.tensor_tensor(out=m2[:], in0=hA[:], in1=hA[:], op=OP.mult)
                cA = sp.tile([P, 512], f16, tag="cA")
                nc.vector.tensor_scalar(
                    out=cA[:], in0=m2[:], scalar1=-2.0, scalar2=1.0,
                    op0=OP.mult, op1=OP.add)

                vc = sp.tile([P, GW], f16, tag="vc")
                vcv = vc[:].rearrange("p (t e) -> p t e", e=TW)
                sAv = sA[:].rearrange("p (t e) -> p t e", e=M)
                cAv = cA[:].rearrange("p (t e) -> p t e", e=M)
                nc.vector.tensor_tensor(out=vcv[:, :, 0:M], in0=Eg, in1=cAv, op=OP.mult)
                nc.gpsimd.tensor_tensor(out=vcv[:, :, M:TW], in0=Eg, in1=sAv, op=OP.mult)

                if stage == 2:
                    nc.scalar.dma_start(
                        out=q_d.ap()[:, 1024 * G:1024 * (G + 1)], in_=vc[:])
                    continue

                for half in range(2):
                    vcT = pp.tile([P, 512], f16, tag="vcT", bufs=2)
                    for t4 in range(4):
                        nc.tensor.transpose(
                            out=vcT[:, 128 * t4:128 * (t4 + 1)],
                            in_=vc[:, 512 * half + 128 * t4: 512 * half + 128 * (t4 + 1)],
                            identity=ident[:])
                    vcTs = sp.tile([P, 512], f16, tag="vcTs")
                    nc.vector.tensor_copy(out=vcTs[:], in_=vcT[:])
                    if stage == 3:
                        nc.scalar.dma_start(
                            out=q_d.ap()[:, 1024 * G + 512 * half:
                                         1024 * G + 512 * (half + 1)],
                            in_=vcTs[:])
                        continue
                    qT = pp.tile([P, 512], f32, tag="qT", bufs=2)
                    nc.tensor.matmul(out=qT[:], lhsT=bdw2[:], rhs=vcTs[:],
                                     start=True, stop=True)
                    qs = sp.tile([P, 512], f16, tag="qs")
                    if half == 0:
                        nc.scalar.activation(out=qs[:], in_=qT[:], func=AF.Copy)
                    else:
                        nc.vector.tensor_copy(out=qs[:], in_=qT[:])
                    nc.scalar.dma_start(
                        out=q_d.ap()[:, 1024 * G + 512 * half: 1024 * G + 512 * (half + 1)],
                        in_=qs[:])

    nc.compile()
    return nc


def _prep_inputs(x: np.ndarray, shuffle_vector: np.ndarray):
    x_bf = np.asarray(x).astype(ml_dtypes.bfloat16)
    tabs = _tables(shuffle_vector)
    in_maps = []
    for n in range(NCORES):
        xc = x_bf[n * RPC:(n + 1) * RPC]                    # [32768, 16]
        xT8 = np.ascontiguousarray(
            xc.reshape(NBLK, BLKC, K).transpose(0, 2, 1).reshape(P, BLKC))
        in_maps.append({"xT8": xT8, **tabs})
    return in_maps


def _decode(q_cores: list, shuffle_vector: np.ndarray) -> np.ndarray:
    # q_d [128, 8192] fp16 per core; partition 32j+dc, col 128*tg + l
    # tile tau = 4*(tg % 8) ... see mapping below
    out = np.empty((B, M), np.complex128)
    row_of = np.empty((4, 64, 128), np.int64)    # [j, tg, l] -> row in core
    for tg in range(64):
        G, t4 = tg // 8, tg % 8
        a, h = G // 2, G % 2
        for j in range(4):
            tau = 4 * t4 + j
            cc, t = tau // 8, tau % 8
            blk = a + 4 * (cc % 2)
            colib = 2048 * h + 1024 * (cc // 2) + 128 * t
            row_of[j, tg, :] = 4096 * blk + colib + np.arange(128)
    for n in range(NCORES):
        q = np.asarray(q_cores[n], np.float64).reshape(4, TW, 64, P)  # j, dc, tg, l
        qc = q[:, 0::2] + 1j * q[:, 1::2]                             # j, e, tg, l
        rows = np.empty((RPC, M), np.complex128)
        ro = row_of.reshape(4, 64 * 128)
        for j in range(4):
            rows[ro[j]] = qc[j].reshape(M, 64 * 128).T
        out[n * RPC:(n + 1) * RPC] = rows
    # coefficients: c_0 = 1, c_16 = q_0 - 1, c_d = q_{16-d} for d=1..15
    out *= np.exp(4.0)              # undo the per-group -1 log-mag offset
    q0 = out[:, 0]
    coeffs = np.empty((B, K + 1), np.complex128)
    coeffs[:, 0] = 1.0
    coeffs[:, 16] = q0 - 1.0
    coeffs[:, 1:16] = out[:, 1:][:, ::-1]
    l2 = np.sqrt(np.sum(np.abs(coeffs) ** 2, axis=-1, keepdims=True))
    return coeffs / l2 * np.sqrt(K + 1)


def kernel(x: np.ndarray, shuffle_vector: np.ndarray) -> np.ndarray:
    global _cached
    x = np.asarray(x)
    assert x.shape == (B, K), x.shape
    if _cached is None:
        _cached = _build_module()
    in_maps = _prep_inputs(x, shuffle_vector)
    res = bass_utils.run_bass_kernel_spmd(
        _cached, in_maps, core_ids=list(range(NCORES)))
    return _decode([res.results[n]["q"] for n in range(NCORES)], shuffle_vector)
